# revision 30
# baseline (speedup 1.0000x reference)
"""Trainium2 Bass kernel for nn_ChimeraNet (encoder -> 10-step Euler RNN -> LN -> readout).

Data-parallel over 8 NeuronCores: each core gets 1024 rows of the batch and a
replicated set of (host-prefolded) weights.

Math (per core, R=1024 rows, D=1024), with u = h/0.2 so the update is
    u_{t+1} = 0.8*u_t + tanh(u_t @ (0.2 W_res) + drive_in),   u_1 = tanh(drive_in)

The recurrent matmul runs in fp8-e4m3 DoubleRow mode: K=256 per instruction at
1 column/cycle -> 2x the f32r/bf16 FLOP rate (measured ~213ns per N=512
instruction, the 157 TF/s peak).  Per step and 512-row slice, each output
m-pair is one PSUM tile [128,1024] built by 2x4 DoubleRow matmuls computing
S*(u @ 0.2*W_res) with S=32 folded into W8 = e4m3(S*0.2*W_res); then
    d  = psum + S*drive_in          (DVE pair-wide add, drive_in kept in f32)
    v  = tanh(d / S)                (ACT pair-wide, scale=1/S)
    u' = 0.8*u + v                  (DVE pair-wide stt, f32r state)
    u8 = e4m3(u')                   (pair-wide cast, alternating DVE/ACT)
All elementwise ops are pair-wide [128, 2, 512] = 1024 elems: DVE ops cost
~0.7us regardless of width up to ~2048, so wide ops crush per-op overhead.
Engine budget per step: PE 13.6us (bottleneck), DVE ~13us, ACT ~11us.

Encoder x @ (W_enc.T W_in) runs in bf16 from a HOST-pre-transposed x.T (no
on-chip transposes).  LayerNorm+readout are folded into a [12, R] matmul
(y rows 0-9, S1 row 10, S2 row 11 appended via ones@(u^2)) followed by a
width-8 stats chain and two stt ops per 128-row tile.
"""

import os
import sys

import numpy as np
import ml_dtypes

try:
    import concourse.bass as bass  # noqa: F401
except ImportError:  # pragma: no cover - fresh grading env without PYTHONPATH
    for p in ("/root/.axon_site", "/root/.axon_site/_ro/trn_rl_repo",
              "/root/.axon_site/_ro/pypackages", "/opt/trn_rl_repo"):
        if os.path.isdir(p) and p not in sys.path:
            sys.path.append(p)
    import concourse.bass as bass

from contextlib import ExitStack

import concourse.tile as tile
from concourse import bacc, bass_utils, mybir
from concourse.masks import make_identity

N_CORES = 8
B = 8192
R = B // N_CORES        # rows per core
D = 1024                # latent dim
KX = 784                # encoder input dim
DT_STEP = 0.2
STEPS = 10
EPS = 1e-5
S = 32.0                # fp8 psum domain scale

F32 = mybir.dt.float32
F32R = mybir.dt.float32r
BF16 = mybir.dt.bfloat16
FP16 = mybir.dt.float16
E4 = mybir.dt.float8e4
AF = mybir.ActivationFunctionType
ALU = mybir.AluOpType
DR = mybir.MatmulPerfMode.DoubleRow

KD = D // 128           # 8 k/m tiles over D
NS = R // 512           # 2 moving-dim slices of 512
KXT = [128] * 6 + [16]  # 784 = 6*128 + 16
NWARM = 4               # PE warmup matmuls while first DMAs land

E4NP = ml_dtypes.float8_e4m3
BF16NP = ml_dtypes.bfloat16


def _build_program():
    nc = bacc.Bacc("TRN2", target_bir_lowering=False, debug=False)

    # host-pre-transposed x: xt[k, p, r] = x[r, k*128+p] (bf16)
    xt = nc.dram_tensor("xt", [len(KXT), 128, R], BF16, kind="ExternalInput").ap()
    wc16 = nc.dram_tensor("wc16", [KX, D], BF16, kind="ExternalInput").ap()
    w8 = nc.dram_tensor("w8", [128, KD, D], E4, kind="ExternalInput").ap()
    bias = nc.dram_tensor("bias", [D], F32, kind="ExternalInput").ap()
    w2a = nc.dram_tensor("w2a", [D, 11], F32, kind="ExternalInput").ap()
    w1 = nc.dram_tensor("w1", [10], F32, kind="ExternalInput").ap()
    b2 = nc.dram_tensor("b2", [10], F32, kind="ExternalInput").ap()
    out = nc.dram_tensor("out", [R, 10], F32, kind="ExternalOutput").ap()

    with tile.TileContext(nc) as tc, ExitStack() as ctx:
        state = ctx.enter_context(tc.tile_pool(name="state", bufs=1))
        consts = ctx.enter_context(tc.tile_pool(name="consts", bufs=1))
        wres_pool = ctx.enter_context(tc.tile_pool(name="wres", bufs=1))

        # persistent SBUF state (all transposed: D on partitions, rows free).
        # u in f32r: full-rate DVE ALU + direct f32r readout matmul in the tail.
        u_sb = [state.tile([128, KD, R], F32R, name=f"u{b}", tag=f"u{b}")
                for b in range(2)]
        u8_sb = [state.tile([128, KD, R], E4, name=f"u8{b}", tag=f"u8{b}")
                 for b in range(2)]
        # d/v hold a single 512-row n-slice: consumed immediately downstream
        d_sb = state.tile([128, KD, 512], F32, name="d", tag="d")
        v_sb = state.tile([128, KD, 512], F32, name="v", tag="v")
        sq_sb = state.tile([128, KD, R], FP16, name="sq", tag="sq")
        din = state.tile([128, KD, R], F32, name="din", tag="din")  # S*drive_in
        w8_sb = wres_pool.tile([128, KD, D], E4, name="w8sb", tag="w8sb")

        with ExitStack() as mmctx:
            # PE warmup: dependency-free fp32 matmuls pull the clock up while
            # the input DMAs are in flight.
            warmctx = ExitStack()
            warm_psum = warmctx.enter_context(
                tc.tile_pool(name="warm", bufs=1, space="PSUM"))
            warm_src = consts.tile([128, 256], F32)
            nc.vector.memset(warm_src, 0.01)
            warm_sb = consts.tile([128, 1], F32)
            for w in range(NWARM):
                wp = warm_psum.tile([128, 512], F32, name=f"warm{w}", tag="wm")
                nc.tensor.matmul(wp[:, :256], lhsT=warm_src[:, :128], rhs=warm_src,
                                 start=True, stop=True)
                if w == NWARM - 1:
                    nc.vector.tensor_copy(warm_sb, wp[:, :1])  # keep-alive

            ident = consts.tile([128, 128], F32)
            make_identity(nc, ident)
            bias_sb = consts.tile([128, KD], F32)
            nc.gpsimd.dma_start(out=bias_sb, in_=bias.rearrange("(m p) -> p m", p=128))

            # W8 arrives on the gpsimd queue while x.T streams on sync.
            nc.gpsimd.dma_start(out=w8_sb, in_=w8)

            # ------------ encoder: drive = x @ W_c + bias (bf16 matmuls) ------
            with ExitStack() as enc:
                xt_pool = enc.enter_context(tc.tile_pool(name="xt", bufs=1))
                wc_pool = enc.enter_context(tc.tile_pool(name="wc", bufs=1))
                eps_pool = enc.enter_context(
                    tc.tile_pool(name="emm", bufs=6, space="PSUM"))

                xt_big = xt_pool.tile([128, len(KXT), R], BF16, name="xt_big")
                wc_sb = [wc_pool.tile([128, D], BF16, name=f"wc{k}", tag=f"wc{k}")
                         for k in range(len(KXT))]
                # interleave x.T and W_c chunk DMAs so the k=0.. accumulation
                # can start as soon as the first chunks land
                for k, kw in enumerate(KXT):
                    nc.sync.dma_start(out=xt_big[:kw, k, :], in_=xt[k, :kw, :])
                    nc.scalar.dma_start(out=wc_sb[k][:kw, :],
                                        in_=wc16[k * 128:k * 128 + kw, :])

                for n in range(NS):
                    sl = slice(n * 512, (n + 1) * 512)
                    for mp in range(KD // 2):
                        mm2 = slice(2 * mp, 2 * mp + 2)
                        for half in range(2):
                            m = 2 * mp + half
                            ps = eps_pool.tile([128, 512], F32,
                                               name=f"eps{n}_{m}", tag="emm")
                            for k, kw in enumerate(KXT):
                                nc.tensor.matmul(
                                    ps,
                                    lhsT=wc_sb[k][:kw, m * 128:(m + 1) * 128],
                                    rhs=xt_big[:kw, k, sl],
                                    start=(k == 0), stop=(k == len(KXT) - 1))
                            # din = S*(x@W_c) + S*bias: S in the ACT scale, the
                            # bias tensor is pre-scaled by S on the host
                            nc.scalar.activation(din[:, m, sl], ps, AF.Identity,
                                                 bias=bias_sb[:, m:m + 1],
                                                 scale=S)
                        # u_1 = v_0 = tanh(drive_in) = tanh(din/S), pair-wide
                        nc.scalar.activation(u_sb[1][:, mm2, sl],
                                             din[:, mm2, sl], AF.Tanh,
                                             scale=1.0 / S)
                        nc.vector.tensor_copy(u8_sb[1][:, mm2, sl],
                                              u_sb[1][:, mm2, sl])
            warmctx.close()

            # ------------ Euler integration loop (fp8 DoubleRow) --------------
            loopctx = ExitStack()
            psum = loopctx.enter_context(
                tc.tile_pool(name="mm", bufs=3, space="PSUM"))

            for s in range(1, STEPS):
                cur, nxt = s % 2, (s + 1) % 2
                for n in range(NS):
                    sl = slice(n * 512, (n + 1) * 512)
                    for mp in range(KD // 2):
                        mm2 = slice(2 * mp, 2 * mp + 2)
                        ps = psum.tile([128, 1024], F32, name=f"ps{s}_{n}_{mp}",
                                       tag="mm")
                        for half in range(2):
                            m = 2 * mp + half
                            psh = ps[:, half * 512:(half + 1) * 512]
                            for k2 in range(KD // 2):
                                nc.tensor.matmul(
                                    psh,
                                    lhsT=w8_sb[:, 2 * k2:2 * k2 + 2,
                                               m * 128:(m + 1) * 128],
                                    rhs=u8_sb[cur][:, 2 * k2:2 * k2 + 2, sl],
                                    start=(k2 == 0), stop=(k2 == KD // 2 - 1),
                                    perf_mode=DR)
                        # d = psum + S*drive_in  (pair-wide DVE add from PSUM)
                        nc.vector.tensor_add(
                            d_sb[:, mm2, :],
                            ps.rearrange("p (two c) -> p two c", two=2),
                            din[:, mm2, sl])
                        # v = tanh(d/S) for both halves in one ACT op
                        nc.scalar.activation(v_sb[:, mm2, :], d_sb[:, mm2, :],
                                             AF.Tanh, scale=1.0 / S)
                        # pair-wide state update on DVE
                        nc.vector.scalar_tensor_tensor(
                            u_sb[nxt][:, mm2, sl], in0=u_sb[cur][:, mm2, sl],
                            scalar=1.0 - DT_STEP, op0=ALU.mult,
                            in1=v_sb[:, mm2, :], op1=ALU.add)
                        if s < STEPS - 1:
                            # e4m3 cast for the next step's matmuls
                            if mp % 2 == 0:
                                nc.vector.tensor_copy(u8_sb[nxt][:, mm2, sl],
                                                      u_sb[nxt][:, mm2, sl])
                            else:
                                nc.scalar.copy(u8_sb[nxt][:, mm2, sl],
                                               u_sb[nxt][:, mm2, sl])
                        else:
                            # last step: u8 dead; LN squares instead (DVE, no
                            # ACT table switch)
                            nc.vector.tensor_mul(sq_sb[:, mm2, sl],
                                                 u_sb[nxt][:, mm2, sl],
                                                 u_sb[nxt][:, mm2, sl])

            loopctx.close()
            uf = u_sb[STEPS % 2]

            # ------------ tail: LN stats + readout ----------------------------
            tail = ctx.enter_context(tc.tile_pool(name="tail", bufs=1))
            tmm = mmctx.enter_context(
                tc.tile_pool(name="tmm", bufs=2, space="PSUM"))
            tp2 = mmctx.enter_context(
                tc.tile_pool(name="tp2", bufs=2, space="PSUM"))

            ones_f32 = tail.tile([128, 1], F32)
            nc.vector.memset(ones_f32, 1.0)
            ones_sb = tail.tile([128, 1], FP16)
            nc.scalar.copy(ones_sb, ones_f32)
            eps_sb = tail.tile([128, 8], F32)
            nc.vector.memset(eps_sb, EPS)
            # w2a = [0.2*W2.T | ones] : readout weights + S1 column
            w2a_sb = tail.tile([128, KD, 11], F32R)
            nc.gpsimd.dma_start(out=w2a_sb,
                                in_=w2a.rearrange("(k p) o -> p k o",
                                                  p=128).bitcast(F32R))
            w1_bc = tail.tile([128, 10], F32)
            nc.gpsimd.dma_start(out=w1_bc, in_=bass.AP(tensor=w1.tensor, offset=w1.offset,
                                                       ap=[[0, 128]] + list(w1.ap)))
            b2_bc = tail.tile([128, 10], F32)
            nc.gpsimd.dma_start(out=b2_bc, in_=bass.AP(tensor=b2.tensor, offset=b2.offset,
                                                       ap=[[0, 128]] + list(b2.ap)))

            # y rows 0-9: W2 @ h.T; row 10: S1 = sum_D u; s2: S2 = sum u^2
            y_sb = tail.tile([11, R], F32)
            s2_sb = tail.tile([1, R], F32)

            yt_all = tail.tile([128, KD, 12], F32)   # transposed per-row stats
            mu_n = tail.tile([128, KD], F32)
            ex2 = tail.tile([128, KD], F32)
            var = tail.tile([128, KD], F32)
            sd = tail.tile([128, KD], F32)
            inv = tail.tile([128, KD], F32)
            qn = tail.tile([128, KD], F32)

            for n in range(NS):
                sl = slice(n * 512, (n + 1) * 512)
                yp = tmm.tile([128, 512], F32, name=f"yp{n}", tag="tmm")
                for k in range(KD):
                    nc.tensor.matmul(yp[:11, :], lhsT=w2a_sb[:, k, :],
                                     rhs=uf[:, k, sl],
                                     start=(k == 0), stop=(k == KD - 1))
                nc.scalar.copy(y_sb[:, sl], yp[:11, :])
                s2 = tmm.tile([128, 512], F32, name=f"s2p{n}", tag="tmm")
                for k in range(KD):
                    nc.tensor.matmul(s2[:1, :], lhsT=ones_sb,
                                     rhs=sq_sb[:, k, sl],
                                     start=(k == 0), stop=(k == KD - 1))
                nc.vector.tensor_copy(s2_sb[:, sl], s2[:1, :])

                for rt in range(n * 4, (n + 1) * 4):
                    rsl = slice(rt * 128, (rt + 1) * 128)
                    yn = tp2.tile([128, 11], F32, name=f"yn{rt}", tag="st")
                    nc.tensor.transpose(yn, y_sb[:, rsl], ident[:11, :11])
                    nc.vector.tensor_copy(yt_all[:, rt, 0:11], yn)
                    p2 = tp2.tile([128, 1], F32, name=f"p2_{rt}", tag="st")
                    nc.tensor.transpose(p2, s2_sb[:, rsl], ident[:1, :1])
                    nc.vector.tensor_copy(yt_all[:, rt, 11:12], p2)

            # width-8 stats chain (one op per stage for all 8 row-tiles)
            nc.scalar.mul(mu_n, yt_all[:, :, 10], -DT_STEP / D)      # -mean(h)
            nc.scalar.mul(ex2, yt_all[:, :, 11], DT_STEP * DT_STEP / D)
            nc.vector.scalar_tensor_tensor(var, in0=mu_n, scalar=-1.0,
                                           op0=ALU.mult, in1=mu_n, op1=ALU.mult)
            nc.vector.tensor_add(var, var, ex2)
            nc.scalar.activation(sd, var, AF.Sqrt, bias=eps_sb[:, 0:1], scale=1.0)
            nc.vector.reciprocal(inv, sd)
            nc.vector.tensor_mul(qn, mu_n, inv)                      # -mu*inv

            for rt in range(8):
                rsl = slice(rt * 128, (rt + 1) * 128)
                # a = qn*w1 + b2 ;  o = inv*y + a
                a = tail.tile([128, 10], F32, name=f"a{rt}", tag="a", bufs=2)
                nc.vector.scalar_tensor_tensor(a, in0=w1_bc,
                                               scalar=qn[:, rt:rt + 1],
                                               op0=ALU.mult, in1=b2_bc,
                                               op1=ALU.add)
                o = tail.tile([128, 10], F32, name=f"o{rt}", tag="o", bufs=2)
                nc.vector.scalar_tensor_tensor(o, in0=yt_all[:, rt, 0:10],
                                               scalar=inv[:, rt:rt + 1],
                                               op0=ALU.mult, in1=a, op1=ALU.add)
                nc.sync.dma_start(out=out[rsl, :], in_=o)

    nc.compile()
    return nc


_NC_CACHE = None


def _get_program():
    global _NC_CACHE
    if _NC_CACHE is None:
        _NC_CACHE = _build_program()
    return _NC_CACHE


def _prepare_in_maps(inputs):
    x = np.asarray(inputs["x"], dtype=np.float32)
    w_enc = np.asarray(inputs["W_enc"], dtype=np.float32)
    w_res = np.asarray(inputs["W_res"], dtype=np.float32)
    w_in = np.asarray(inputs["W_in"], dtype=np.float32)
    bias = np.asarray(inputs["bias"], dtype=np.float32)
    ln_g = np.asarray(inputs["ln_g"], dtype=np.float32)
    ln_b = np.asarray(inputs["ln_b"], dtype=np.float32)
    w_out = np.asarray(inputs["W_out"], dtype=np.float32)
    b_out = np.asarray(inputs["b_out"], dtype=np.float32)

    w_c = (w_enc.T.astype(np.float64) @ w_in.astype(np.float64)).astype(np.float32)
    w2 = w_out * ln_g[None, :]                       # [10, D]
    # fp8 stationary weights in the S-scaled domain: [128, KD, D],
    # element (p, ks, m) = S*0.2*W_res[ks*128+p, m]
    w8 = (S * DT_STEP * w_res).astype(E4NP).reshape(KD, 128, D).transpose(1, 0, 2)
    w2a = np.empty((D, 11), np.float32)
    w2a[:, :10] = DT_STEP * w2.T                     # readout: gives W2 @ h.T
    w2a[:, 10] = 1.0                                 # S1 column: sum_D u
    w1v = w2.sum(axis=1).astype(np.float32)
    b2v = (w_out.astype(np.float64) @ ln_b.astype(np.float64)
           + b_out.astype(np.float64)).astype(np.float32)

    shared = {
        "wc16": np.ascontiguousarray(w_c.astype(BF16NP)),
        "w8": np.ascontiguousarray(w8),
        "bias": np.ascontiguousarray(S * bias),   # din lives in the S-domain
        "w2a": np.ascontiguousarray(w2a),
        "w1": np.ascontiguousarray(w1v),
        "b2": np.ascontiguousarray(b2v),
    }
    # host-transposed x per core: xt[k, p, r] = x[r, k*128+p] in bf16
    x16 = x.astype(BF16NP)
    in_maps = []
    for c in range(N_CORES):
        xc = x16[c * R:(c + 1) * R, :]               # [R, 784]
        xp = np.zeros((len(KXT), 128, R), dtype=BF16NP)
        xcT = np.ascontiguousarray(xc.T)             # [784, R]
        for k, kw in enumerate(KXT):
            xp[k, :kw, :] = xcT[k * 128:k * 128 + kw, :]
        m = dict(shared)
        m["xt"] = np.ascontiguousarray(xp)
        in_maps.append(m)
    return in_maps


def run(inputs, trace=False, tmpdir=None):
    """Run on 8 NeuronCores; returns (out [8192,10], BassKernelResults)."""
    nc = _get_program()
    in_maps = _prepare_in_maps(inputs)
    res = bass_utils.run_bass_kernel_spmd(
        nc, in_maps, core_ids=list(range(N_CORES)), trace=trace, tmpdir=tmpdir)
    outs = [np.asarray(r["out"]) for r in res.results]
    return np.concatenate(outs, axis=0), res


def kernel(**inputs):
    out, _ = run(inputs, trace=False)
    return out
